# revision 9
# baseline (speedup 1.0000x reference)
"""Expert-parallel MoE kernel for Trainium2 (8 NeuronCores).

Problem: top-2 MoE, N=8192 tokens, D=1024, H=4096, E=8 experts.
Strategy (expert parallel):
  - Host: compute gating (logits -> top-k -> softmax) exactly as the
    reference does (CPU jax, fp32), dispatch tokens to their experts.
  - Core e holds expert e's weights; it runs a 2-layer MLP over the
    tokens routed to it (padded to a fixed capacity C), plus the
    combine() row-renormalization:
        y = (relu(x @ w1 + b1) @ w2 + b2)
        y_scaled = y * (gate * ||x||) / (||y|| + 1e-8)
  - Host: scatter-add per-expert outputs back to the [N, D] result.

Device kernel layout (per core, all matmuls bf16 with fp32 PSUM accum):
  xT   [D, C]  bf16  tokens transposed (D-major so lhsT/rhs tiles are natural)
  w1   [D, H]  bf16
  b1   [H]     f32
  w2   [H, D]  bf16
  b2   [D]     f32
  sc   [C]     f32   per-token gate * ||x||
  y    [C, D]  f32   scaled expert output

Loop structure: token blocks of <=512; layer 1 produces hT [H, R] in SBUF
(H on partitions) streaming w1 column chunks; layer 2 accumulates
out[R, D] in PSUM over the 32 H-tiles with w2 resident in SBUF; the
epilogue does bias add, row sum-of-squares (ACT Square + accum), sqrt,
reciprocal and the final scale, then DMAs the block out.
"""

import os
import sys

import numpy as np

if "/opt/trn_rl_repo" not in sys.path:
    sys.path.insert(0, "/opt/trn_rl_repo")

import ml_dtypes

N, D, H, E = 8192, 1024, 4096, 8
P = 128
BLK = 512  # max token block

_nc_cache = {}


def _build_nc(C):
    """Build the per-core Bass program for capacity C (multiple of 128)."""
    from contextlib import ExitStack

    import concourse.bass as bass
    import concourse.mybir as mybir
    import concourse.tile as tile
    from concourse import bacc

    f32 = mybir.dt.float32
    bf16 = mybir.dt.bfloat16
    AF = mybir.ActivationFunctionType

    n_k = D // P   # 8  K-tiles for layer 1
    n_h = H // P   # 32 H-tiles
    HC = 512       # w1 column-chunk width (4 h-tiles)

    nc = bacc.Bacc(trn_type="TRN2", num_devices=E)
    xT = nc.dram_tensor("xT", [D, C], bf16, kind="ExternalInput")
    w1 = nc.dram_tensor("w1", [D, H], bf16, kind="ExternalInput")
    b1 = nc.dram_tensor("b1", [H], f32, kind="ExternalInput")
    w2 = nc.dram_tensor("w2", [H, D], bf16, kind="ExternalInput")
    b2 = nc.dram_tensor("b2", [D], f32, kind="ExternalInput")
    sc = nc.dram_tensor("sc", [C], f32, kind="ExternalInput")
    y = nc.dram_tensor("y", [C, D], f32, kind="ExternalOutput")

    xT_t = xT.ap().rearrange("(k p) c -> p k c", p=P)
    w1_t = w1.ap().rearrange("(k p) h -> p k h", p=P)
    w2_t = w2.ap().rearrange("(h p) d -> p h d", p=P)
    b1_t = b1.ap().rearrange("(h p) -> p h", p=P)
    sc_t = sc.ap().rearrange("(m p) -> p m", p=P)
    y_t = y.ap().rearrange("(o p) d -> p o d", p=P)

    blocks = []
    off = 0
    while off < C:
        r = min(BLK, C - off)
        blocks.append((off, r))
        off += r

    with tile.TileContext(nc) as tc, ExitStack() as ctx:
        singles = ctx.enter_context(tc.tile_pool(name="singles", bufs=1))
        xpool = ctx.enter_context(tc.tile_pool(name="xpool", bufs=3))
        w1pool = ctx.enter_context(tc.tile_pool(name="w1pool", bufs=3))
        hpool = ctx.enter_context(tc.tile_pool(name="hpool", bufs=1))
        stpool = ctx.enter_context(tc.tile_pool(name="stpool", bufs=2))
        sqpool = ctx.enter_context(tc.tile_pool(name="sqpool", bufs=2))
        smpool = ctx.enter_context(tc.tile_pool(name="smpool", bufs=4))
        psh = ctx.enter_context(tc.tile_pool(name="psh", bufs=2, space="PSUM"))
        pso = ctx.enter_context(tc.tile_pool(name="pso", bufs=2, space="PSUM"))

        # --- preamble: constants ---
        # Queue discipline: the x/w1 stream lives on the sync-engine HWDGE
        # queue and must never wait behind big transfers, so the 8 MB w2
        # load goes on the vector-engine queue and y outputs go on the
        # scalar-engine queue.
        b1_sb = singles.tile([P, n_h], f32)
        nc.sync.dma_start(out=b1_sb, in_=b1_t)
        b2_sb = singles.tile([P, D], f32)
        b2_bcast = bass.AP(tensor=b2.ap().tensor, offset=b2.ap().offset,
                           ap=[[0, P], *b2.ap().ap])
        nc.gpsimd.dma_start(out=b2_sb, in_=b2_bcast)
        sc_sb = singles.tile([P, C // P], f32)
        nc.sync.dma_start(out=sc_sb, in_=sc_t)
        w2_sb = singles.tile([P, n_h, D], bf16)
        nc.scalar.dma_start(out=w2_sb, in_=w2_t)

        for (B, R) in blocks:
            m_tiles = R // P
            xt = xpool.tile([P, n_k, BLK], bf16, tag="xt", name="xt")[:, :, :R]
            nc.sync.dma_start(out=xt, in_=xT_t[:, :, B:B + R])

            # --- layer 1: hT[h, tokens] = relu(x @ w1 + b1), H on partitions ---
            hT = hpool.tile([P, n_h, BLK], bf16, tag="hT", name="hT")[:, :, :R]
            for hc in range(H // HC):
                w1c = w1pool.tile([P, n_k, HC], bf16, tag="w1c")
                nc.sync.dma_start(out=w1c, in_=w1_t[:, :, hc * HC:(hc + 1) * HC])
                for hs in range(HC // P):
                    h = hc * (HC // P) + hs
                    ps = psh.tile([P, BLK], f32, tag="ph", name="ph")[:, :R]
                    for k in range(n_k):
                        nc.tensor.matmul(
                            ps,
                            lhsT=w1c[:, k, hs * P:(hs + 1) * P],
                            rhs=xt[:, k, :],
                            start=(k == 0),
                            stop=(k == n_k - 1),
                        )
                    nc.scalar.activation(
                        out=hT[:, h, :], in_=ps, func=AF.Relu,
                        bias=b1_sb[:, h:h + 1], scale=1.0,
                    )

            # --- layer 2: out[tokens, D] accumulated over h; epilogue ---
            stage = stpool.tile([P, BLK // P, D], f32, tag="stage", name="stage")[:, :m_tiles, :]
            q = smpool.tile([P, BLK // P], f32, tag="q", name="q")[:, :m_tiles]
            for m in range(m_tiles):
                po = pso.tile([P, D], f32, tag="po")
                for h in range(n_h):
                    for n2 in range(2):
                        nc.tensor.matmul(
                            po[:, n2 * 512:(n2 + 1) * 512],
                            lhsT=hT[:, h, m * P:(m + 1) * P],
                            rhs=w2_sb[:, h, n2 * 512:(n2 + 1) * 512],
                            start=(h == 0),
                            stop=(h == n_h - 1),
                        )
                # stage = out + b2 ; q[:, m] = sum(stage^2)
                nc.vector.tensor_add(out=stage[:, m, :], in0=po, in1=b2_sb)
                sq = sqpool.tile([P, D], f32, tag="sq")
                nc.scalar.activation(
                    out=sq, in_=stage[:, m, :], func=AF.Square,
                    accum_out=q[:, m:m + 1],
                )

            # f = sc / (sqrt(q) + 1e-8); y = stage * f
            qs = smpool.tile([P, BLK // P], f32, tag="qs", name="qs")[:, :m_tiles]
            nc.scalar.activation(out=qs, in_=q, func=AF.Sqrt)
            nc.vector.tensor_scalar_add(out=qs, in0=qs, scalar1=1e-8)
            nc.vector.reciprocal(out=qs, in_=qs)
            f = smpool.tile([P, BLK // P], f32, tag="f", name="f")[:, :m_tiles]
            nc.vector.tensor_mul(out=f, in0=qs,
                                 in1=sc_sb[:, B // P:B // P + m_tiles])
            for m in range(m_tiles):
                nc.vector.tensor_scalar_mul(
                    out=stage[:, m, :], in0=stage[:, m, :],
                    scalar1=f[:, m:m + 1],
                )
            nc.scalar.dma_start(out=y_t[:, B // P:B // P + m_tiles, :], in_=stage)

    nc.compile()
    return nc


def _get_nc(C):
    if C not in _nc_cache:
        _nc_cache[C] = _build_nc(C)
    return _nc_cache[C]


LAST_EXEC_NS = None
LAST_TRACE = None


def _install_axon_ntff_hook():
    """Register antenv.axon_hooks shim driving NTFF capture via the axon .so.

    The agent image's antenv package lacks axon_hooks, so concourse's
    trace=True path degrades. Replicates trn_boot._ntff_profile_via_ctypes.
    """
    import contextlib
    import ctypes
    import types

    if "antenv.axon_hooks" in sys.modules:
        return
    lib = ctypes.CDLL("/opt/axon/libaxon_pjrt.so")
    if not hasattr(lib, "axon_start_nrt_profile"):
        return
    lib.axon_start_nrt_profile.argtypes = [ctypes.POINTER(ctypes.c_int64),
                                           ctypes.c_size_t]
    lib.axon_start_nrt_profile.restype = ctypes.c_int64
    lib.axon_stop_nrt_profile.argtypes = [ctypes.c_char_p]
    lib.axon_stop_nrt_profile.restype = ctypes.c_int64

    @contextlib.contextmanager
    def _hook(output_dir, device_ids):
        import jax
        jax.devices()
        if device_ids:
            ids = (ctypes.c_int64 * len(device_ids))(*device_ids)
            rc = lib.axon_start_nrt_profile(ids, len(device_ids))
        else:
            rc = lib.axon_start_nrt_profile(None, 0)
        if rc != 0:
            raise RuntimeError(f"axon_start_nrt_profile rc={rc}")
        try:
            yield
        finally:
            n = lib.axon_stop_nrt_profile(str(output_dir).encode())
            print(f"ntff capture: {n} file(s) -> {output_dir}", file=sys.stderr)

    mod = types.ModuleType("antenv.axon_hooks")
    mod.get_axon_ntff_profile_hook = lambda: _hook
    sys.modules["antenv.axon_hooks"] = mod
    import antenv
    antenv.axon_hooks = mod


def _gating(x, w_gate, k):
    """Top-k gating computed exactly like the reference (CPU jax, fp32)."""
    import jax
    import jax.numpy as jnp

    cpu = jax.devices("cpu")[0]
    with jax.default_device(cpu):
        xj = jnp.asarray(x)
        logits = xj @ jnp.asarray(w_gate)
        top_vals, top_idx = jax.lax.top_k(logits, k)
        top_gates = jax.nn.softmax(top_vals, axis=-1)
        init_norm = jnp.linalg.norm(xj, axis=-1)
        return (np.asarray(top_idx), np.asarray(top_gates, np.float32),
                np.asarray(init_norm, np.float32))


def kernel(x, w_gate, w1, b1, w2, b2, k):
    from concourse.bass_utils import run_bass_kernel_spmd

    x = np.asarray(x, np.float32)
    w_gate = np.asarray(w_gate, np.float32)
    w1 = np.asarray(w1, np.float32)
    b1 = np.asarray(b1, np.float32)
    w2 = np.asarray(w2, np.float32)
    b2 = np.asarray(b2, np.float32)
    k = int(np.asarray(k))
    n, d = x.shape
    e = w_gate.shape[1]

    top_idx, top_gates, init_norm = _gating(x, w_gate, k)

    idxs, scs = [], []
    for ei in range(e):
        tok, slot = np.nonzero(top_idx == ei)
        idxs.append(tok)
        scs.append(top_gates[tok, slot] * init_norm[tok])

    maxc = max(len(t) for t in idxs)
    C = max(((maxc + P - 1) // P) * P, P)
    nc = _get_nc(C)

    bf16 = ml_dtypes.bfloat16
    in_maps = []
    for ei in range(e):
        tok = idxs[ei]
        xTe = np.zeros((d, C), bf16)
        xTe[:, :len(tok)] = x[tok].T
        sce = np.zeros((C,), np.float32)
        sce[:len(tok)] = scs[ei]
        in_maps.append({
            "xT": xTe,
            "w1": np.ascontiguousarray(w1[ei]).astype(bf16),
            "b1": np.ascontiguousarray(b1[ei]),
            "w2": np.ascontiguousarray(w2[ei]).astype(bf16),
            "b2": np.ascontiguousarray(b2[ei]),
            "sc": sce,
        })

    trace = bool(int(os.environ.get("MOE_TRACE", "0")))
    kwargs = {}
    if trace:
        _install_axon_ntff_hook()
        tdir = os.environ.get("MOE_TRACE_DIR")
        if tdir:
            os.makedirs(tdir, exist_ok=True)
            kwargs["tmpdir"] = tdir
        kwargs["trace_cores"] = [0]
    res = run_bass_kernel_spmd(
        nc, in_maps, core_ids=list(range(E)), trace=trace, **kwargs,
    )
    global LAST_EXEC_NS, LAST_TRACE
    LAST_EXEC_NS = res.exec_time_ns
    LAST_TRACE = res.instructions_and_trace
    if res.exec_time_ns is not None:
        print(f"HW exec time: {res.exec_time_ns} ns", file=sys.stderr)

    y = np.zeros((n, d), np.float32)
    for ei in range(e):
        tok = idxs[ei]
        y[tok] += res.results[ei]["y"][:len(tok)]
    return y
